# revision 11
# baseline (speedup 1.0000x reference)
"""Trainium2 Bass kernel for a 12-head self-attention block.

Reference computation (per batch b of 8):
    qkv = x @ w_qkv                      # (1024, 2304)
    q, k, v per head (12 heads, d=64)
    attn = softmax(q k^T / sqrt(64))
    ctx  = attn @ v                      # (1024, 768)
    y    = ctx @ w_proj + b_proj

Sharding: data parallel over the batch dim — batch b runs on core b.
Each core gets the full weights and its own x slice; no collectives.

Host-side prep (inside kernel(), so it is self-contained): x is
transposed and cast to bf16 (x^T is the only layout the device ever
needs — it is the contraction-side operand of every GEMM), and the
weights are cast to bf16 and pre-arranged into the exact SBUF tile
layouts, halving the input DMA bytes and removing all on-device
transposes and casts.

Per-core dataflow:
  - All matmul operands bf16 (1 col/cycle on the PE); accumulation is
    fp32 in PSUM, softmax statistics stay fp32.
  - Input DMAs are staged: the x^T tiles and pair-0 q/k weights get
    the full HBM bandwidth first (the DMA engines fair-share, so
    issuing everything at once makes the first-needed data arrive
    last); later weights are released via tiny gate-copies once x^T
    has landed.
  - q^T/k^T come from qk^T = W_qk^T @ X^T (keys/queries on the free
    axis) so the S matmul needs no transposes.
  - V is computed in natural (token, feature) layout with a column of
    ones per head: attn @ v then yields the softmax denominator in
    PSUM partition 64 for free.
  - Attention runs in 512-query halves.  Per key tile, both heads of
    a pair write one 2-bank PSUM tile, consumed by a single 1024-wide
    exp on ScalarE (the ScalarE is the attention-phase roofline, so
    halving its per-instruction overhead matters; softmax max
    subtraction is skipped: logits ~N(0,1)).  P·V runs two key tiles
    behind S (software pipeline) so the exp latency never stalls the
    PE, and the NEXT pair's qk^T matmuls are interleaved one-per-slot
    as PE filler between S and P·V.
  - Normalize per query-half: fast reciprocal of the denominator row,
    gpsimd partition-broadcast, one multiply -> ctx^T per head pair.
  - Projection ladder: each output tile's first 5 contraction steps
    are emitted ahead of older tiles' final steps (two PSUM rings =
    ladder depth 3), hiding the last normalization.  Bias is added by
    the DVE during the PSUM->SBUF copy.
  - Matmul output chunks never cross a PSUM bank (512 fp32) boundary.
"""

import numpy as np

N = 1024          # tokens per batch (32*32)
C = 768           # model dim
NH = 12           # heads
NP = NH // 2      # head pairs
D = 64            # head dim
NT = N // 128     # 8 token tiles
KC = C // 128     # 6 contraction tiles
SCALE = D ** -0.5
NCORES = 8

_CACHE = {}


def _build_nc():
    import concourse.bass as bass
    import concourse.tile as tile
    from concourse import bacc, mybir

    F32 = mybir.dt.float32
    BF16 = mybir.dt.bfloat16
    Exp = mybir.ActivationFunctionType.Exp

    nc = bacc.Bacc(None, target_bir_lowering=False)
    xT_d = nc.declare_dram_parameter("xT", [KC, 128, N], BF16, isOutput=False)
    wqk_d = nc.declare_dram_parameter("wqk", [NP, 128, KC, 256], BF16,
                                      isOutput=False)
    wv_d = nc.declare_dram_parameter("wv", [128, KC, C], BF16,
                                     isOutput=False)
    wp_d = nc.declare_dram_parameter("wp", [2, 128, KC, 384], BF16,
                                     isOutput=False)
    bproj = nc.declare_dram_parameter("b_proj", [1, C], F32, isOutput=False)
    y = nc.declare_dram_parameter("y", [N, C], F32, isOutput=True)

    with tile.TileContext(nc) as tc:
        from contextlib import ExitStack

        with ExitStack() as ctx:
            persist = ctx.enter_context(tc.tile_pool(name="persist", bufs=1))
            xTs = [persist.tile([128, N], BF16, name=f"xT{kc}")
                   for kc in range(KC)]
            wqks = [persist.tile([128, KC, 256], BF16, name=f"wqk{j}")
                    for j in range(NP)]
            wv = persist.tile([128, KC, C], BF16)
            wps = [persist.tile([128, KC, 384], BF16, name=f"wp{i}")
                   for i in range(2)]
            V = persist.tile([128, NT, NH, D + 2], BF16)   # v + ones col
            ctxTs = [persist.tile([128, N], BF16, name=f"ctxT{j}")
                     for j in range(NP)]
            ones_f32 = persist.tile([128, 2 * NH], F32)
            bias_sb = persist.tile([1, C], F32)
            bias_bc = persist.tile([128, C], F32)

            nc.vector.memset(ones_f32[:], 1.0)
            for _t in range(NT):
                # write ones in pairs (4-byte chunks): lone 2-byte strided
                # writes are not safe on the compute engines
                nc.any.tensor_copy(
                    out=V[:, _t, :, D:D + 2],
                    in_=ones_f32[:].rearrange("p (h two) -> p h two", two=2),
                )

            # ---- wave-1 input DMAs: xT + pair-0 q/k weights --------------
            for kc in range(KC):
                eng = nc.sync if kc % 2 == 0 else nc.scalar
                eng.dma_start(out=xTs[kc][:], in_=xT_d[kc])
            nc.sync.dma_start(out=wqks[0][:], in_=wqk_d[0])
            nc.gpsimd.dma_start(out=bias_sb[:], in_=bproj[:])
            nc.gpsimd.partition_broadcast(
                bias_bc[:], bias_sb[0:1, :], channels=128
            )

            # ---- wave-2/3: gated behind the wave-1 transfers -------------
            # (tiny DVE copy into each target tile makes its DMA wait)
            def gate(tile_ap, src):
                nc.vector.tensor_copy(out=tile_ap, in_=src)

            gate(wv[0:1, 0, 0:2], xTs[5][0:1, 0:2])
            gate(wqks[1][0:1, 0, 0:2], xTs[5][0:1, 0:2])
            nc.scalar.dma_start(out=wv[:], in_=wv_d[:, :, :])
            nc.sync.dma_start(out=wqks[1][:], in_=wqk_d[1])
            for j in range(2, NP):
                gate(wqks[j][0:1, 0, 0:2], wv[0:1, 0, 0:2])
                eng = nc.sync if j % 2 == 0 else nc.scalar
                eng.dma_start(out=wqks[j][:], in_=wqk_d[j])
            for i in range(2):
                gate(wps[i][0:1, 0, 0:2], wv[0:1, 0, 0:2])
                eng = nc.scalar if i == 0 else nc.sync
                eng.dma_start(out=wps[i][:], in_=wp_d[i])

            # PSUM: S ring 2x4KB + O^T halves 2x2KB + qk/proj ring 2x2KB
            psS = ctx.enter_context(tc.tile_pool(name="psS", bufs=2,
                                                 space="PSUM"))
            psO = ctx.enter_context(tc.tile_pool(name="psO", bufs=2,
                                                 space="PSUM"))
            psQ = ctx.enter_context(tc.tile_pool(name="psQ", bufs=2,
                                                 space="PSUM"))

            qkpool = ctx.enter_context(tc.tile_pool(name="qk", bufs=3))
            ptpool = ctx.enter_context(tc.tile_pool(name="pt", bufs=6))
            bcpool = ctx.enter_context(tc.tile_pool(name="bc", bufs=4))
            oupool = ctx.enter_context(tc.tile_pool(name="ou", bufs=4))
            outpool = ctx.enter_context(tc.tile_pool(name="out", bufs=3))

            def qk_units(j, burst):
                """qk^T for pair j: emit now (burst) or return closures."""
                qT = qkpool.tile([128, N], BF16, tag="qT", name=f"qT{j}")
                kT = qkpool.tile([128, N], BF16, tag="kT", name=f"kT{j}")
                units = []
                for dst, woff in ((qT, 0), (kT, 128)):
                    for c in range(2):
                        sl = slice(c * 512, (c + 1) * 512)

                        def mk_mm(woff_, sl_, kc_):
                            def emit(ps):
                                nc.tensor.matmul(
                                    ps[:],
                                    lhsT=wqks[j][:, kc_,
                                                 woff_:woff_ + 128],
                                    rhs=xTs[kc_][:, sl_],
                                    start=(kc_ == 0),
                                    stop=(kc_ == KC - 1),
                                )
                            return emit

                        def mk_cp(dst_, sl_):
                            def emit(ps):
                                nc.vector.tensor_copy(out=dst_[:, sl_],
                                                      in_=ps[:])
                            return emit

                        units.append(("alloc", None))
                        for kc in range(KC):
                            units.append(("mm", mk_mm(woff, sl, kc)))
                        units.append(("cp", mk_cp(dst, sl)))
                if burst:
                    ps = None
                    for kind, fn in units:
                        if kind == "alloc":
                            ps = psQ.tile([128, 512], F32, tag="q")
                        else:
                            fn(ps)
                    return qT, kT, None
                return qT, kT, units

            class Filler:
                def __init__(self, units):
                    self.units = list(units) if units else []
                    self.i = 0
                    self.ps = None

                def step(self, n):
                    for _ in range(n):
                        if self.i >= len(self.units):
                            return
                        kind, fn = self.units[self.i]
                        if kind == "alloc":
                            self.ps = psQ.tile([128, 512], F32, tag="q")
                        else:
                            fn(self.ps)
                        self.i += 1

                def finish(self):
                    self.step(len(self.units) - self.i)

            # ---- qk pair 0, V, qk pair 1 (PE bursts) ---------------------
            qk_tiles = [qk_units(0, burst=True)]

            for t in range(NT):
                ps = psS.tile([128, 2, 512], F32, tag="s")
                for ci, sl in enumerate((slice(0, 512), slice(512, C))):
                    w_ = sl.stop - sl.start
                    for kc in range(KC):
                        nc.tensor.matmul(
                            ps[:, ci, 0:w_],
                            lhsT=xTs[kc][:, t * 128:(t + 1) * 128],
                            rhs=wv[:, kc, sl],
                            start=(kc == 0),
                            stop=(kc == KC - 1),
                        )
                nc.vector.tensor_copy(
                    out=V[:, t, 0:8, 0:D],
                    in_=ps[:, 0, :].rearrange("p (h d) -> p h d", d=D),
                )
                nc.vector.tensor_copy(
                    out=V[:, t, 8:NH, 0:D],
                    in_=ps[:, 1, 0:256].rearrange("p (h d) -> p h d", d=D),
                )

            qk_tiles.append(qk_units(1, burst=True))

            # ---- attention: pair j, query-half c, P·V two tiles behind ---
            LAG = 2
            for j in range(NP):
                qT, kT = qk_tiles[j][0], qk_tiles[j][1]
                if j + 2 <= NP - 1:
                    nqT, nkT, units = qk_units(j + 2, burst=False)
                    qk_tiles.append((nqT, nkT, None))
                    filler = Filler(units)
                else:
                    filler = Filler(None)

                for c in range(2):
                    qsl = slice(c * 512, (c + 1) * 512)
                    OTs = [psO.tile([D + 1, 512], F32, tag="ot",
                                    name=f"ot{j}_{c}_{hh}")
                           for hh in range(2)]
                    pts = []

                    def emit_pv(t):
                        for hh in range(2):
                            nc.tensor.matmul(
                                OTs[hh][:],
                                lhsT=V[:, t, 2 * j + hh, 0:D + 1],
                                rhs=pts[t][:, hh, :],
                                start=(t == 0),
                                stop=(t == NT - 1),
                            )

                    for t in range(NT):
                        S2 = psS.tile([128, 2, 512], F32, tag="s")
                        for hh in range(2):
                            pb = hh * 64
                            nc.tensor.matmul(
                                S2[:, hh, :],
                                lhsT=kT[pb:pb + 64, t * 128:(t + 1) * 128],
                                rhs=qT[pb:pb + 64, qsl],
                                start=True,
                                stop=True,
                            )
                        pT = ptpool.tile([128, 2, 512], BF16, tag="pt")
                        nc.scalar.activation(
                            out=pT[:], in_=S2[:], func=Exp, scale=SCALE
                        )
                        pts.append(pT)
                        filler.step(2)
                        if t >= LAG:
                            emit_pv(t - LAG)
                    for t in range(NT - LAG, NT):
                        emit_pv(t)
                    # normalize this query-half for both heads
                    for hh in range(2):
                        pb = hh * 64
                        ou = oupool.tile([D + 1, 512], F32, tag="ou")
                        nc.vector.tensor_copy(out=ou[:], in_=OTs[hh][:])
                        den = bcpool.tile([1, 512], F32, tag="den")
                        nc.vector.tensor_copy(out=den[:],
                                              in_=ou[D:D + 1, :])
                        bc = bcpool.tile([64, 512], F32, tag="bc")
                        nc.vector.reciprocal_approx_fast(
                            out=bc[0:1, :], in_=den[:]
                        )
                        nc.gpsimd.partition_broadcast(
                            bc[:], bc[0:1, :], channels=64
                        )
                        nc.vector.tensor_mul(
                            out=ctxTs[j][pb:pb + 64, qsl],
                            in0=ou[0:D, :], in1=bc[:],
                        )
                filler.finish()

            # ---- projection ladder: y = ctx @ W_proj + b -----------------
            def pj_head(nt, cch, i):
                if i % 2 == 0:
                    ps_full = psS.tile([128, 2, 512], F32, tag="s",
                                       name=f"pj{nt}_{cch}")
                    ps = ps_full[:, 0, 0:384]
                else:
                    ps_full = psQ.tile([128, 512], F32, tag="q",
                                       name=f"pj{nt}_{cch}")
                    ps = ps_full[:, 0:384]
                for kc in range(KC - 1):
                    nc.tensor.matmul(
                        ps,
                        lhsT=ctxTs[kc][:, nt * 128:(nt + 1) * 128],
                        rhs=wps[cch][:, kc, :],
                        start=(kc == 0),
                        stop=False,
                    )
                return ps

            def pj_finish(nt, cch, ps):
                kc = KC - 1
                sl = slice(cch * 384, (cch + 1) * 384)
                nc.tensor.matmul(
                    ps,
                    lhsT=ctxTs[kc][:, nt * 128:(nt + 1) * 128],
                    rhs=wps[cch][:, kc, :],
                    start=False,
                    stop=True,
                )
                ob = outpool.tile([128, 384], F32, tag="ob")
                nc.vector.tensor_add(
                    out=ob[:], in0=ps, in1=bias_bc[:, sl]
                )
                nc.sync.dma_start(
                    out=y[nt * 128:(nt + 1) * 128, sl.start:sl.start + 192],
                    in_=ob[:, 0:192],
                )
                nc.scalar.dma_start(
                    out=y[nt * 128:(nt + 1) * 128, sl.start + 192:sl.stop],
                    in_=ob[:, 192:384],
                )

            tiles = [(nt, cch) for nt in range(NT) for cch in range(2)]
            pending = []
            for i, (nt, cch) in enumerate(tiles):
                pending.append((nt, cch, pj_head(nt, cch, i)))
                if len(pending) == 3:
                    pj_finish(*pending.pop(0))
            while pending:
                pj_finish(*pending.pop(0))

    nc.finalize()
    return nc


def _get_nc():
    if "nc" not in _CACHE:
        _CACHE["nc"] = _build_nc()
    return _CACHE["nc"]


def _make_in_maps(x, w_qkv, w_proj, b_proj):
    import ml_dtypes

    BF = ml_dtypes.bfloat16
    B = x.shape[0]
    xb = x.reshape(B, N, C).astype(np.float32)
    w_qkv = np.asarray(w_qkv, dtype=np.float32)
    w_proj = np.asarray(w_proj, dtype=np.float32)
    bp = np.ascontiguousarray(b_proj.reshape(1, C).astype(np.float32))

    # weight tiles in the exact SBUF layouts ([partition, kc, cols])
    wq3 = w_qkv.reshape(KC, 128, 3 * C)       # [kc, p, col]
    wqk = np.empty((NP, 128, KC, 256), dtype=BF)
    for j in range(NP):
        blk = np.concatenate(
            [wq3[:, :, j * 128:(j + 1) * 128],
             wq3[:, :, C + j * 128:C + (j + 1) * 128]], axis=2
        )  # [kc, p, 256]
        wqk[j] = blk.transpose(1, 0, 2).astype(BF)
    wv = np.ascontiguousarray(
        wq3[:, :, 2 * C:3 * C].transpose(1, 0, 2)
    ).astype(BF)                                # [p, kc, 768]
    wp3 = w_proj.reshape(KC, 128, C).transpose(1, 0, 2)   # [p, kc, col]
    wp = np.empty((2, 128, KC, 384), dtype=BF)
    for i in range(2):
        wp[i] = wp3[:, :, i * 384:(i + 1) * 384].astype(BF)

    maps = []
    for b in range(B):
        xT = np.ascontiguousarray(
            xb[b].T.reshape(KC, 128, N)
        ).astype(BF)
        maps.append({
            "xT": xT, "wqk": wqk, "wv": wv, "wp": wp, "b_proj": bp,
        })
    return maps


def _run(in_maps, **kwargs):
    from concourse.bass_utils import run_bass_kernel_spmd

    nc = _get_nc()
    return run_bass_kernel_spmd(
        nc, in_maps, core_ids=list(range(NCORES)), **kwargs
    )


def kernel(x, w_qkv, w_proj, b_proj):
    B, H, W, _ = x.shape
    res = _run(_make_in_maps(x, w_qkv, w_proj, b_proj))
    out = np.stack([res.results[b]["y"] for b in range(B)])
    return out.reshape(B, H, W, C).astype(np.float32)


# revision 12
# speedup vs baseline: 1.1041x; 1.1041x over previous
"""Trainium2 Bass kernel for a 12-head self-attention block.

Reference computation (per batch b of 8):
    qkv = x @ w_qkv                      # (1024, 2304)
    q, k, v per head (12 heads, d=64)
    attn = softmax(q k^T / sqrt(64))
    ctx  = attn @ v                      # (1024, 768)
    y    = ctx @ w_proj + b_proj

Sharding: data parallel over the batch dim — batch b runs on core b.
Each core gets the full weights and its own x slice; no collectives.

Host-side prep (inside kernel(), so it is self-contained): x is
transposed and cast to bf16 (x^T is the only layout the device ever
needs — it is the contraction-side operand of every GEMM), and the
weights are cast to bf16 and pre-arranged into the exact SBUF tile
layouts, halving the input DMA bytes and removing all on-device
transposes and casts.

Schedule: the ScalarE exp throughput (~1.3 ns/elem over 12.6M softmax
elements, ~130us) is the attention-phase roofline, so ALL other PE
work is threaded through the attention windows as filler so the PE
never becomes the constraint:
  - wave-1 DMAs (x^T, pair-0 q/k weights, W_v) get full HBM bandwidth;
    later weights are gated behind them (the DMA engines fair-share,
    so issuing everything at once makes first-needed data arrive last).
  - lead-in: qk^T for pair 0, then V = X W_v (with a ones column per
    head: attn @ v then yields the softmax denominator in PSUM
    partition 64 for free).
  - attention pair j: per key tile both heads' S^T go into one 2-bank
    PSUM tile -> a single 1024-wide exp -> P·V two key tiles behind
    (exp latency hidden).  Interleaved per slot: next pair's qk^T
    matmuls (2/slot) and one projection partial-step
    (y_acc += ctx^T_{j-1} W_proj rows, accumulated in SBUF by the DVE
    since PSUM cannot hold 16 open accumulators).
  - normalize per query-half: fast reciprocal of the denominator row,
    gpsimd partition-broadcast, one multiply -> ctx^T.
  - tail: only the last pair's projection step + bias + stores.
  - Matmul output chunks never cross a PSUM bank (512 fp32) boundary.
"""

import numpy as np

N = 1024          # tokens per batch (32*32)
C = 768           # model dim
NH = 12           # heads
NP = NH // 2      # head pairs
D = 64            # head dim
NT = N // 128     # 8 token tiles
KC = C // 128     # 6 contraction tiles
SCALE = D ** -0.5
NCORES = 8

_CACHE = {}


def _build_nc():
    import concourse.bass as bass
    import concourse.tile as tile
    from concourse import bacc, mybir

    F32 = mybir.dt.float32
    BF16 = mybir.dt.bfloat16
    Exp = mybir.ActivationFunctionType.Exp

    nc = bacc.Bacc(None, target_bir_lowering=False)
    xT_d = nc.declare_dram_parameter("xT", [KC, 128, N], BF16, isOutput=False)
    wqk_d = nc.declare_dram_parameter("wqk", [NP, 128, KC, 256], BF16,
                                      isOutput=False)
    wv_d = nc.declare_dram_parameter("wv", [128, KC, C], BF16,
                                     isOutput=False)
    wp_d = nc.declare_dram_parameter("wp", [2, 128, KC, 384], BF16,
                                     isOutput=False)
    bproj = nc.declare_dram_parameter("b_proj", [1, C], F32, isOutput=False)
    y = nc.declare_dram_parameter("y", [N, C], F32, isOutput=True)

    with tile.TileContext(nc) as tc:
        from contextlib import ExitStack

        with ExitStack() as ctx:
            persist = ctx.enter_context(tc.tile_pool(name="persist", bufs=1))
            xTs = [persist.tile([128, N], BF16, name=f"xT{kc}")
                   for kc in range(KC)]
            wqks = [persist.tile([128, KC, 256], BF16, name=f"wqk{j}")
                    for j in range(NP)]
            wv = persist.tile([128, KC, C], BF16)
            wps = [persist.tile([128, KC, 384], BF16, name=f"wp{i}")
                   for i in range(2)]
            V = persist.tile([128, NT, NH, D + 2], BF16)   # v + ones col
            ctxTs = [persist.tile([128, N], BF16, name=f"ctxT{j}")
                     for j in range(NP)]
            accs = [persist.tile([128, 384], F32, name=f"acc{i}")
                    for i in range(2 * NT)]                # y accumulators
            ones_f32 = persist.tile([128, 2 * NH], F32)
            bias_sb = persist.tile([1, C], F32)
            bias_bc = persist.tile([128, C], F32)

            nc.vector.memset(ones_f32[:], 1.0)
            for _t in range(NT):
                # write ones in pairs (4-byte chunks): lone 2-byte strided
                # writes are not safe on the compute engines
                nc.any.tensor_copy(
                    out=V[:, _t, :, D:D + 2],
                    in_=ones_f32[:].rearrange("p (h two) -> p h two", two=2),
                )

            # ---- wave-1 input DMAs: xT, pair-0 q/k, W_v ------------------
            for kc in range(KC):
                eng = nc.sync if kc % 2 == 0 else nc.scalar
                eng.dma_start(out=xTs[kc][:], in_=xT_d[kc])
            nc.sync.dma_start(out=wqks[0][:], in_=wqk_d[0])
            nc.scalar.dma_start(out=wv[:, 0:3, :], in_=wv_d[:, 0:3, :])
            nc.sync.dma_start(out=wv[:, 3:KC, :], in_=wv_d[:, 3:KC, :])
            nc.gpsimd.dma_start(out=bias_sb[:], in_=bproj[:])
            nc.gpsimd.partition_broadcast(
                bias_bc[:], bias_sb[0:1, :], channels=128
            )

            # ---- wave-2: later weights, gated behind wave-1 --------------
            # (tiny DVE copy into each target tile makes its DMA wait)
            for j in range(1, NP):
                nc.vector.tensor_copy(out=wqks[j][0:1, 0, 0:2],
                                      in_=xTs[5][0:1, 0:2])
                eng = nc.sync if j % 2 == 0 else nc.scalar
                eng.dma_start(out=wqks[j][:], in_=wqk_d[j])
            for i in range(2):
                nc.vector.tensor_copy(out=wps[i][0:1, 0, 0:2],
                                      in_=xTs[5][0:1, 0:2])
                eng = nc.scalar if i == 0 else nc.sync
                eng.dma_start(out=wps[i][:], in_=wp_d[i])

            # PSUM: S ring 2x4KB + O^T halves 2x2KB + qk 2KB + proj 2KB
            psS = ctx.enter_context(tc.tile_pool(name="psS", bufs=2,
                                                 space="PSUM"))
            psO = ctx.enter_context(tc.tile_pool(name="psO", bufs=2,
                                                 space="PSUM"))
            psQ = ctx.enter_context(tc.tile_pool(name="psQ", bufs=1,
                                                 space="PSUM"))
            psP = ctx.enter_context(tc.tile_pool(name="psP", bufs=1,
                                                 space="PSUM"))

            qkpool = ctx.enter_context(tc.tile_pool(name="qk", bufs=2))
            ptpool = ctx.enter_context(tc.tile_pool(name="pt", bufs=6))
            bcpool = ctx.enter_context(tc.tile_pool(name="bc", bufs=4))
            oupool = ctx.enter_context(tc.tile_pool(name="ou", bufs=4))

            def qk_units(j, burst):
                """qk^T for pair j: emit now (burst) or return closures."""
                qT = qkpool.tile([128, N], BF16, tag="qT", name=f"qT{j}")
                kT = qkpool.tile([128, N], BF16, tag="kT", name=f"kT{j}")
                units = []
                for dst, woff in ((qT, 0), (kT, 128)):
                    for c in range(2):
                        sl = slice(c * 512, (c + 1) * 512)

                        def mk_mm(woff_, sl_, kc_):
                            def emit(ps):
                                nc.tensor.matmul(
                                    ps[:],
                                    lhsT=wqks[j][:, kc_,
                                                 woff_:woff_ + 128],
                                    rhs=xTs[kc_][:, sl_],
                                    start=(kc_ == 0),
                                    stop=(kc_ == KC - 1),
                                )
                            return emit

                        def mk_cp(dst_, sl_):
                            def emit(ps):
                                nc.vector.tensor_copy(out=dst_[:, sl_],
                                                      in_=ps[:])
                            return emit

                        units.append(("alloc", None))
                        for kc in range(KC):
                            units.append(("mm", mk_mm(woff, sl, kc)))
                        units.append(("cp", mk_cp(dst, sl)))
                if burst:
                    ps = None
                    for kind, fn in units:
                        if kind == "alloc":
                            ps = psQ.tile([128, 512], F32, tag="q")
                        else:
                            fn(ps)
                    return qT, kT, None
                return qT, kT, units

            class Filler:
                def __init__(self, units):
                    self.units = list(units) if units else []
                    self.i = 0
                    self.ps = None

                def step(self, n):
                    for _ in range(n):
                        if self.i >= len(self.units):
                            return
                        kind, fn = self.units[self.i]
                        if kind == "alloc":
                            self.ps = psQ.tile([128, 512], F32, tag="q")
                        else:
                            fn(self.ps)
                        self.i += 1

                def finish(self):
                    self.step(len(self.units) - self.i)

            def pj_step(i, kc):
                """One projection partial: accs[i] += ctx^T_kc W_p rows."""
                nt, cch = i // 2, i % 2
                ps = psP.tile([128, 384], F32, tag="p")
                nc.tensor.matmul(
                    ps[:],
                    lhsT=ctxTs[kc][:, nt * 128:(nt + 1) * 128],
                    rhs=wps[cch][:, kc, :],
                    start=True,
                    stop=True,
                )
                sl = slice(cch * 384, (cch + 1) * 384)
                if kc == 0:
                    nc.vector.tensor_add(out=accs[i][:], in0=ps[:],
                                         in1=bias_bc[:, sl])
                else:
                    nc.vector.tensor_add(out=accs[i][:], in0=ps[:],
                                         in1=accs[i][:])

            # ---- lead-in: qk pair 0, then V ------------------------------
            qk_tiles = [qk_units(0, burst=True)]

            for t in range(NT):
                ps = psS.tile([128, 2, 512], F32, tag="s")
                for ci, sl in enumerate((slice(0, 512), slice(512, C))):
                    w_ = sl.stop - sl.start
                    for kc in range(KC):
                        nc.tensor.matmul(
                            ps[:, ci, 0:w_],
                            lhsT=xTs[kc][:, t * 128:(t + 1) * 128],
                            rhs=wv[:, kc, sl],
                            start=(kc == 0),
                            stop=(kc == KC - 1),
                        )
                nc.vector.tensor_copy(
                    out=V[:, t, 0:8, 0:D],
                    in_=ps[:, 0, :].rearrange("p (h d) -> p h d", d=D),
                )
                nc.vector.tensor_copy(
                    out=V[:, t, 8:NH, 0:D],
                    in_=ps[:, 1, 0:256].rearrange("p (h d) -> p h d", d=D),
                )

            # ---- attention: pair j, query-half c, P·V two tiles behind ---
            LAG = 2
            for j in range(NP):
                qT, kT = qk_tiles[j][0], qk_tiles[j][1]
                if j + 1 <= NP - 1:
                    nqT, nkT, units = qk_units(j + 1, burst=False)
                    qk_tiles.append((nqT, nkT, None))
                    filler = Filler(units)
                else:
                    filler = Filler(None)
                pj = list(range(2 * NT)) if j >= 1 else []

                for c in range(2):
                    qsl = slice(c * 512, (c + 1) * 512)
                    OTs = [psO.tile([D + 1, 512], F32, tag="ot",
                                    name=f"ot{j}_{c}_{hh}")
                           for hh in range(2)]
                    pts = []

                    def emit_pv(t):
                        for hh in range(2):
                            nc.tensor.matmul(
                                OTs[hh][:],
                                lhsT=V[:, t, 2 * j + hh, 0:D + 1],
                                rhs=pts[t][:, hh, :],
                                start=(t == 0),
                                stop=(t == NT - 1),
                            )

                    for t in range(NT):
                        s = c * NT + t
                        S2 = psS.tile([128, 2, 512], F32, tag="s")
                        for hh in range(2):
                            pb = hh * 64
                            nc.tensor.matmul(
                                S2[:, hh, :],
                                lhsT=kT[pb:pb + 64, t * 128:(t + 1) * 128],
                                rhs=qT[pb:pb + 64, qsl],
                                start=True,
                                stop=True,
                            )
                        pT = ptpool.tile([128, 2, 512], BF16, tag="pt")
                        nc.scalar.activation(
                            out=pT[:], in_=S2[:], func=Exp, scale=SCALE
                        )
                        pts.append(pT)
                        filler.step(2)
                        if pj and s >= 3 and (s - 3) < len(pj):
                            pj_step(pj[s - 3], j - 1)
                        if t >= LAG:
                            emit_pv(t - LAG)
                    for t in range(NT - LAG, NT):
                        emit_pv(t)
                    # normalize this query-half for both heads
                    for hh in range(2):
                        pb = hh * 64
                        ou = oupool.tile([D + 1, 512], F32, tag="ou")
                        nc.vector.tensor_copy(out=ou[:], in_=OTs[hh][:])
                        den = bcpool.tile([1, 512], F32, tag="den")
                        nc.vector.tensor_copy(out=den[:],
                                              in_=ou[D:D + 1, :])
                        bc = bcpool.tile([64, 512], F32, tag="bc")
                        nc.vector.reciprocal_approx_fast(
                            out=bc[0:1, :], in_=den[:]
                        )
                        nc.gpsimd.partition_broadcast(
                            bc[:], bc[0:1, :], channels=64
                        )
                        nc.vector.tensor_mul(
                            out=ctxTs[j][pb:pb + 64, qsl],
                            in0=ou[0:D, :], in1=bc[:],
                        )
                filler.finish()
                for i in range(13, 2 * NT):
                    if pj:
                        pj_step(pj[i], j - 1)

            # ---- tail: last projection step + stores ---------------------
            for i in range(2 * NT):
                nt, cch = i // 2, i % 2
                kc = NP - 1
                pool = psP if i % 2 == 0 else psQ
                ps = pool.tile([128, 384 if i % 2 == 0 else 512], F32,
                               tag="p" if i % 2 == 0 else "q")
                ps = ps[:, 0:384]
                nc.tensor.matmul(
                    ps,
                    lhsT=ctxTs[kc][:, nt * 128:(nt + 1) * 128],
                    rhs=wps[cch][:, kc, :],
                    start=True,
                    stop=True,
                )
                nc.vector.tensor_add(out=accs[i][:], in0=ps,
                                     in1=accs[i][:])
                sl = slice(cch * 384, (cch + 1) * 384)
                nc.sync.dma_start(
                    out=y[nt * 128:(nt + 1) * 128, sl.start:sl.start + 192],
                    in_=accs[i][:, 0:192],
                )
                nc.scalar.dma_start(
                    out=y[nt * 128:(nt + 1) * 128, sl.start + 192:sl.stop],
                    in_=accs[i][:, 192:384],
                )

    nc.finalize()
    return nc


def _get_nc():
    if "nc" not in _CACHE:
        _CACHE["nc"] = _build_nc()
    return _CACHE["nc"]


def _make_in_maps(x, w_qkv, w_proj, b_proj):
    import ml_dtypes

    BF = ml_dtypes.bfloat16
    B = x.shape[0]
    xb = x.reshape(B, N, C).astype(np.float32)
    w_qkv = np.asarray(w_qkv, dtype=np.float32)
    w_proj = np.asarray(w_proj, dtype=np.float32)
    bp = np.ascontiguousarray(b_proj.reshape(1, C).astype(np.float32))

    # weight tiles in the exact SBUF layouts ([partition, kc, cols])
    wq3 = w_qkv.reshape(KC, 128, 3 * C)       # [kc, p, col]
    wqk = np.empty((NP, 128, KC, 256), dtype=BF)
    for j in range(NP):
        blk = np.concatenate(
            [wq3[:, :, j * 128:(j + 1) * 128],
             wq3[:, :, C + j * 128:C + (j + 1) * 128]], axis=2
        )  # [kc, p, 256]
        wqk[j] = blk.transpose(1, 0, 2).astype(BF)
    wv = np.ascontiguousarray(
        wq3[:, :, 2 * C:3 * C].transpose(1, 0, 2)
    ).astype(BF)                                # [p, kc, 768]
    wp3 = w_proj.reshape(KC, 128, C).transpose(1, 0, 2)   # [p, kc, col]
    wp = np.empty((2, 128, KC, 384), dtype=BF)
    for i in range(2):
        wp[i] = wp3[:, :, i * 384:(i + 1) * 384].astype(BF)

    maps = []
    for b in range(B):
        xT = np.ascontiguousarray(
            xb[b].T.reshape(KC, 128, N)
        ).astype(BF)
        maps.append({
            "xT": xT, "wqk": wqk, "wv": wv, "wp": wp, "b_proj": bp,
        })
    return maps


def _run(in_maps, **kwargs):
    from concourse.bass_utils import run_bass_kernel_spmd

    nc = _get_nc()
    return run_bass_kernel_spmd(
        nc, in_maps, core_ids=list(range(NCORES)), **kwargs
    )


def kernel(x, w_qkv, w_proj, b_proj):
    B, H, W, _ = x.shape
    res = _run(_make_in_maps(x, w_qkv, w_proj, b_proj))
    out = np.stack([res.results[b]["y"] for b in range(B)])
    return out.reshape(B, H, W, C).astype(np.float32)
